# revision 11
# baseline (speedup 1.0000x reference)
"""Series decomposition: depthwise moving-average (box W=25, replicate pad)
+ remainder, data-parallel over batch across 8 NeuronCores.

HBM traffic is compressed to 4 bytes/element (host pre-scales x by 1/W and
casts fp16; device emits only the fp16 trend; host forms remainder = x -
trend from fp32 x). At that traffic the DVE sliding-window scan (~1.9
cyc/elem — ALU feedback latency) becomes the bottleneck, so the work is
split across two engine pipelines running concurrently:

- DVE path (D_DVE row-tiles of 128): replicate-padded tile + one
  tensor_tensor_scan  s[i] = s[i-1] + xp[i+12] - xp[i-13]  (fp32 state,
  fp16 in/out).
- PE path (remaining R_PE rows, fed TRANSPOSED by the host as xt[L, R_PE]):
  32 aligned seq-chunks of 128 partitions; per chunk the 25-tap window sum
  is 3 accumulating band matmuls (prev-tile tail / self / next-tile head)
  into PSUM, replicate-clamp folded into the first/last chunk weights;
  the Act engine evicts PSUM fp32 -> fp16. Output stays transposed
  (trendt[L, R_PE]); the host transposes it back.

Engine budget/core: DVE ~8.3us x D_DVE, PE ~1.3us + Act ~1.4us per chunk,
DMA ~2.4us/MB over ~33.6MB.
"""

import numpy as np

import concourse.bacc as bacc
import concourse.bass as bass
import concourse.mybir as mybir
from concourse.bass_utils import run_bass_kernel_spmd
from concourse.tile import TileContext

B, C, L, W = 32, 512, 4096, 25
PAD = W // 2  # 12
NCORES = 8
ROWS = (B // NCORES) * C  # 2048 rows per core
P = 128
LPAD = PAD + 1  # 13 left-pad cols (extra col feeds the scan's subtract lag)
OFF = 32  # x lands 64B-aligned inside the padded tile
XALLOC = 4160  # ring slots stay 64B multiples

D_DVE = 8  # row-tiles on the DVE scan path
R_PE = ROWS - D_DVE * P  # rows on the PE band-matmul path
NCH = L // P  # 32 aligned seq-chunks

FP32 = mybir.dt.float32
FP16 = mybir.dt.float16

WNAMES = ("w_self", "w_self0", "w_self31", "w_prev", "w_next")
WSHAPES = dict(w_self=(P, P), w_self0=(P, P), w_self31=(P, P),
               w_prev=(P, P), w_next=(24, P))


def band_weights():
    """Window-tap-count matrices (input pre-scaled by 1/W -> small exact
    integers). w_prev is stored full-height (taps in rows 116..127) because
    matmul lhsT/rhs must share a base partition in {0,32,64}."""
    w_self = np.zeros((P, P), np.float32)
    for o in range(P):
        for j in range(max(o - PAD, 0), min(o + PAD + 1, P)):
            w_self[j, o] += 1.0
    w_self0 = w_self.copy()
    for o in range(PAD):
        w_self0[0, o] += PAD - o
    w_self31 = w_self.copy()
    for o in range(P - PAD, P):
        w_self31[P - 1, o] += o + PAD - (P - 1)
    w_prev = np.zeros((P, P), np.float32)
    for o in range(PAD):
        for q in range(o + 116, P):
            w_prev[q, o] += 1.0
    w_next = np.zeros((24, P), np.float32)
    for o in range(P - PAD, P):
        for q in range(0, o - 115):
            w_next[q, o] += 1.0
    return {k: np.ascontiguousarray(v.astype(np.float16)) for k, v in dict(
        w_self=w_self, w_self0=w_self0, w_self31=w_self31,
        w_prev=w_prev, w_next=w_next).items()}


def _emit_dve_tile(nc, pool, x, trend, i, l):
    lo = OFF - LPAD
    rsl = slice(i * P, (i + 1) * P)
    xp = pool.tile([P, XALLOC], FP16, tag="xp")
    nc.sync.dma_start(out=xp[:, OFF : OFF + l], in_=x[rsl, :])
    nc.vector.tensor_copy(
        out=xp[:, lo:OFF], in_=xp[:, OFF : OFF + 1].to_broadcast((P, LPAD))
    )
    nc.vector.tensor_copy(
        out=xp[:, OFF + l : OFF + l + PAD],
        in_=xp[:, OFF + l - 1 : OFF + l].to_broadcast((P, PAD)),
    )
    init = pool.tile([P, 1], FP32, tag="init")
    nc.vector.tensor_reduce(
        out=init[:, 0:1], in_=xp[:, lo : lo + W],
        axis=mybir.AxisListType.X, op=mybir.AluOpType.add,
    )
    t = pool.tile([P, l], FP16, tag="t")
    nc.vector.tensor_tensor_scan(
        out=t[:, :],
        data0=xp[:, lo + W : lo + W + l],
        data1=xp[:, lo : lo + l],
        initial=init[:, 0:1],
        op0=mybir.AluOpType.add,
        op1=mybir.AluOpType.subtract,
    )
    nc.sync.dma_start(out=trend[rsl, :], in_=t[:, :])


def _emit_pe_chunk_mms(nc, ps, off, wsb, half, k, xcur, xprev, xnext):
    """Band matmuls for seq-chunk k into ps[:, off : off+1024]."""
    for h in range(2):
        reg = ps[:, off + h * 512 : off + h * 512 + half]
        first, last = k == 0, k == NCH - 1
        wself = wsb["w_self0" if first else ("w_self31" if last else "w_self")]
        ops = []
        if not first:
            ops.append((wsb["w_prev"][64:P, :],
                        xprev[64:P, h * half : (h + 1) * half]))
        ops.append((wself[:, :], xcur[:, h * half : (h + 1) * half]))
        if not last:
            ops.append((wsb["w_next"][:, :],
                        xnext[0:24, h * half : (h + 1) * half]))
        for j, (wt, rv) in enumerate(ops):
            nc.tensor.matmul(reg, wt, rv, start=(j == 0), stop=(j == len(ops) - 1))


def _emit_pe_pair(nc, pool, psum, wsb, tt, rows, half, j, xtiles):
    """Two seq-chunks (2j, 2j+1) share one 4-bank PSUM tile; one Act evict
    and one 3D-AP out-DMA cover both. Evict + out-DMA run on the Activation
    engine (its own HWDGE stream), so a pair waiting on its matmuls never
    head-of-line blocks the SP queue."""
    ps = psum.tile([P, 2048], FP32, tag="ps", bufs=2)
    for c in range(2):
        k = 2 * j + c
        _emit_pe_chunk_mms(
            nc, ps, c * 1024, wsb, half, k,
            xtiles[k],
            xtiles[k - 1] if k > 0 else None,
            xtiles[k + 1] if k < NCH - 1 else None,
        )
    ts = pool.tile([P, 2 * rows], FP16, tag="ts")
    if half == 512:
        nc.scalar.copy(out=ts[:, :], in_=ps[:, :])
    else:
        for c in range(2):
            for h in range(2):
                nc.scalar.copy(
                    out=ts[:, (2 * c + h) * half : (2 * c + h + 1) * half],
                    in_=ps[:, c * 1024 + h * 512 : c * 1024 + h * 512 + half],
                )
    nc.scalar.dma_start(
        out=tt[2 * j * P : (2 * j + 2) * P, :].rearrange("(a p) b -> p a b", p=P),
        in_=ts[:, :],
    )


def build_nc(d=D_DVE, repeats=1):
    """Hybrid kernel for one core: d DVE scan tiles + (ROWS - 128d) PE rows.
    repeats>1 re-runs the sweep inside one NEFF for timing harnesses."""
    r_pe = ROWS - d * P
    half = r_pe // 2
    assert half <= 512 and r_pe % 2 == 0
    nc = bacc.Bacc(trn_type="TRN2")
    x = nc.dram_tensor("x", [max(d * P, 1), L], FP16, kind="ExternalInput")
    trend = nc.dram_tensor("trend", [max(d * P, 1), L], FP16,
                           kind="ExternalOutput")
    if r_pe:
        xt = nc.dram_tensor("xt", [L, r_pe], FP16, kind="ExternalInput")
        wdr = {n: nc.dram_tensor(n, list(WSHAPES[n]), FP16,
                                 kind="ExternalInput") for n in WNAMES}
        tt = nc.dram_tensor("trendt", [L, r_pe], FP16, kind="ExternalOutput")

    with TileContext(nc) as tc:
        with tc.tile_pool(name="wpool", bufs=1) as wpool, \
             tc.tile_pool(name="dpool", bufs=6) as dpool, \
             tc.tile_pool(name="ppool", bufs=4) as ppool, \
             tc.tile_pool(name="psum", bufs=4,
                          space=bass.MemorySpace.PSUM) as psum:
            wsb = {}
            if r_pe:
                for n in WNAMES:
                    wt = wpool.tile(list(WSHAPES[n]), FP16, tag=f"t_{n}")
                    nc.sync.dma_start(out=wt[:, :], in_=wdr[n][:, :])
                    wsb[n] = wt
            for rep in range(repeats):
                # All 32 PE chunk tiles stay resident (they fit in SBUF), so
                # PE matmuls never stall the SP queue on input. Their loads
                # are interleaved with the DVE tiles so the DVE path starts
                # immediately; PE compute waits on semaphores, not the queue.
                xtiles = []
                npair = NCH // 2 if r_pe else 0
                nstep = max(d, npair)
                pairs_done = 0

                def _load_pe(upto):
                    while r_pe and len(xtiles) < min(upto, NCH):
                        k = len(xtiles)
                        xtile = ppool.tile([P, r_pe], FP16, tag="xt", bufs=NCH)
                        nc.sync.dma_start(
                            out=xtile[:, :], in_=xt[k * P : (k + 1) * P, :]
                        )
                        xtiles.append(xtile)

                for i in range(nstep):
                    if i < d:
                        _load_pe((i + 1) * NCH // max(d, 1))
                        _emit_dve_tile(nc, dpool, x, trend, i, L)
                    _load_pe(NCH if i >= d - 1 else 0)
                    while r_pe and pairs_done < min((i + 1) * npair // nstep
                                                    if nstep > d else i + 1,
                                                    npair):
                        j = pairs_done
                        if len(xtiles) >= min(2 * j + 3, NCH):
                            _emit_pe_pair(nc, ppool, psum, wsb, tt, r_pe,
                                          half, j, xtiles)
                            pairs_done += 1
                        else:
                            break
                while r_pe and pairs_done < npair:
                    _emit_pe_pair(nc, ppool, psum, wsb, tt, r_pe, half,
                                  pairs_done, xtiles)
                    pairs_done += 1
    nc.finalize()
    return nc


def _probe_devices():
    """Touch every NeuronCore with a trivial computation to clear stale
    device state from a previous client."""
    try:
        import jax
        import jax.numpy as jnp

        for dev in jax.devices():
            y = jax.device_put(np.ones((4, 4), np.float32), dev)
            jnp.sum(y).block_until_ready()
    except Exception:
        pass


def kernel(x, weight):
    x = np.ascontiguousarray(np.asarray(x), dtype=np.float32)
    scale = float(np.asarray(weight).reshape(-1)[0])
    xs = (x.reshape(NCORES, ROWS, L) * scale).astype(np.float16)
    wts = band_weights()
    nc = build_nc()
    nd = D_DVE * P
    in_maps = []
    for c in range(NCORES):
        m = {"x": np.ascontiguousarray(xs[c, :nd])}
        if R_PE:
            m["xt"] = np.ascontiguousarray(xs[c, nd:].T)
            m.update(wts)
        in_maps.append(m)
    _probe_devices()
    out = None
    for attempt in range(3):
        try:
            out = run_bass_kernel_spmd(nc, in_maps, core_ids=list(range(NCORES)))
            break
        except Exception:
            if attempt == 2:
                raise
            try:
                import jax

                jax.clear_backends()
            except Exception:
                pass
            _probe_devices()
    trend = np.empty((NCORES, ROWS, L), np.float32)
    for c in range(NCORES):
        trend[c, :nd] = out.results[c]["trend"]
        if R_PE:
            trend[c, nd:] = out.results[c]["trendt"].T
    trend = trend.reshape(B, C, L)
    remainder = x.reshape(B, C, L) - trend
    return trend, remainder


# revision 14
# speedup vs baseline: 1.0464x; 1.0464x over previous
"""Series decomposition: depthwise moving-average (box W=25, replicate pad)
+ remainder, data-parallel over batch across 8 NeuronCores.

HBM traffic is compressed to 4 bytes/element (host pre-scales x by 1/W and
casts fp16; device emits only the fp16 trend; host forms remainder = x -
trend from fp32 x). At that traffic the DVE sliding-window scan (~1.9
cyc/elem — ALU feedback latency) becomes the bottleneck, so the work is
split across two engine pipelines running concurrently:

- DVE path (D_DVE row-tiles of 128): replicate-padded tile + one
  tensor_tensor_scan  s[i] = s[i-1] + xp[i+12] - xp[i-13]  (fp32 state,
  fp16 in/out).
- PE path (remaining R_PE rows, fed TRANSPOSED by the host as xt[L, R_PE]):
  32 aligned seq-chunks of 128 partitions; per chunk the 25-tap window sum
  is 3 accumulating band matmuls (prev-tile tail / self / next-tile head)
  into PSUM, replicate-clamp folded into the first/last chunk weights;
  the Act engine evicts PSUM fp32 -> fp16. Output stays transposed
  (trendt[L, R_PE]); the host transposes it back.

Engine budget/core: DVE ~8.3us x D_DVE, PE ~1.3us + Act ~1.4us per chunk,
DMA ~2.4us/MB over ~33.6MB.
"""

import numpy as np

import concourse.bacc as bacc
import concourse.bass as bass
import concourse.mybir as mybir
from concourse.bass_utils import run_bass_kernel_spmd
from concourse.tile import TileContext

B, C, L, W = 32, 512, 4096, 25
PAD = W // 2  # 12
NCORES = 8
ROWS = (B // NCORES) * C  # 2048 rows per core
P = 128
LPAD = PAD + 1  # 13 left-pad cols (extra col feeds the scan's subtract lag)
OFF = 32  # x lands 64B-aligned inside the padded tile
XALLOC = 4160  # ring slots stay 64B multiples

D_DVE = 8  # row-tiles on the DVE scan path
R_PE = ROWS - D_DVE * P  # rows on the PE band-matmul path
NCH = L // P  # 32 aligned seq-chunks

FP32 = mybir.dt.float32
FP16 = mybir.dt.float16

WNAMES = ("w_self", "w_self0", "w_self31", "w_prev", "w_next")
WSHAPES = dict(w_self=(P, P), w_self0=(P, P), w_self31=(P, P),
               w_prev=(P, P), w_next=(24, P))


def band_weights():
    """Window-tap-count matrices (input pre-scaled by 1/W -> small exact
    integers). w_prev is stored full-height (taps in rows 116..127) because
    matmul lhsT/rhs must share a base partition in {0,32,64}."""
    w_self = np.zeros((P, P), np.float32)
    for o in range(P):
        for j in range(max(o - PAD, 0), min(o + PAD + 1, P)):
            w_self[j, o] += 1.0
    w_self0 = w_self.copy()
    for o in range(PAD):
        w_self0[0, o] += PAD - o
    w_self31 = w_self.copy()
    for o in range(P - PAD, P):
        w_self31[P - 1, o] += o + PAD - (P - 1)
    w_prev = np.zeros((P, P), np.float32)
    for o in range(PAD):
        for q in range(o + 116, P):
            w_prev[q, o] += 1.0
    w_next = np.zeros((24, P), np.float32)
    for o in range(P - PAD, P):
        for q in range(0, o - 115):
            w_next[q, o] += 1.0
    return {k: np.ascontiguousarray(v.astype(np.float16)) for k, v in dict(
        w_self=w_self, w_self0=w_self0, w_self31=w_self31,
        w_prev=w_prev, w_next=w_next).items()}


def _emit_dve_tile(nc, pool, x, trend, i, l):
    lo = OFF - LPAD
    rsl = slice(i * P, (i + 1) * P)
    xp = pool.tile([P, XALLOC], FP16, tag="xp")
    nc.sync.dma_start(out=xp[:, OFF : OFF + l], in_=x[rsl, :])
    nc.vector.tensor_copy(
        out=xp[:, lo:OFF], in_=xp[:, OFF : OFF + 1].to_broadcast((P, LPAD))
    )
    nc.vector.tensor_copy(
        out=xp[:, OFF + l : OFF + l + PAD],
        in_=xp[:, OFF + l - 1 : OFF + l].to_broadcast((P, PAD)),
    )
    init = pool.tile([P, 1], FP32, tag="init")
    nc.vector.tensor_reduce(
        out=init[:, 0:1], in_=xp[:, lo : lo + W],
        axis=mybir.AxisListType.X, op=mybir.AluOpType.add,
    )
    t = pool.tile([P, l], FP16, tag="t")
    nc.vector.tensor_tensor_scan(
        out=t[:, :],
        data0=xp[:, lo + W : lo + W + l],
        data1=xp[:, lo : lo + l],
        initial=init[:, 0:1],
        op0=mybir.AluOpType.add,
        op1=mybir.AluOpType.subtract,
    )
    nc.sync.dma_start(out=trend[rsl, :], in_=t[:, :])


def _emit_pe_chunk_mms(nc, ps, off, wsb, half, k, cur, prev, nxt):
    """Band matmuls for seq-chunk k into ps[:, off : off+1024]. cur/prev/nxt
    are (tile, col_base) pairs so paired-load tiles can host two chunks."""
    for h in range(2):
        reg = ps[:, off + h * 512 : off + h * 512 + half]
        first, last = k == 0, k == NCH - 1
        wself = wsb["w_self0" if first else ("w_self31" if last else "w_self")]
        ops = []
        if not first:
            tp, bp = prev
            ops.append((wsb["w_prev"][64:P, :],
                        tp[64:P, bp + h * half : bp + (h + 1) * half]))
        tc_, bc = cur
        ops.append((wself[:, :], tc_[:, bc + h * half : bc + (h + 1) * half]))
        if not last:
            tn, bn = nxt
            ops.append((wsb["w_next"][:, :],
                        tn[0:24, bn + h * half : bn + (h + 1) * half]))
        for j, (wt, rv) in enumerate(ops):
            nc.tensor.matmul(reg, wt, rv, start=(j == 0), stop=(j == len(ops) - 1))


def _emit_pe_pair(nc, pool, psum, wsb, tt, rows, half, j, chunkref):
    """Two seq-chunks (2j, 2j+1) share one 4-bank PSUM tile; one Act evict
    and one 3D-AP out-DMA cover both. Evict + out-DMA run on the Activation
    engine (its own HWDGE stream), so a pair waiting on its matmuls never
    head-of-line blocks the SP queue."""
    ps = psum.tile([P, 2048], FP32, tag="ps", bufs=2)
    for c in range(2):
        k = 2 * j + c
        _emit_pe_chunk_mms(
            nc, ps, c * 1024, wsb, half, k,
            chunkref(k),
            chunkref(k - 1) if k > 0 else None,
            chunkref(k + 1) if k < NCH - 1 else None,
        )
    ts = pool.tile([P, 2 * rows], FP16, tag="ts")
    if half == 512:
        nc.scalar.copy(out=ts[:, :], in_=ps[:, :])
    else:
        for c in range(2):
            for h in range(2):
                nc.scalar.copy(
                    out=ts[:, (2 * c + h) * half : (2 * c + h + 1) * half],
                    in_=ps[:, c * 1024 + h * 512 : c * 1024 + h * 512 + half],
                )
    nc.scalar.dma_start(
        out=tt[2 * j * P : (2 * j + 2) * P, :].rearrange("(a p) b -> p a b", p=P),
        in_=ts[:, :],
    )


def build_nc(d=D_DVE, repeats=1, pair_in=True):
    """Hybrid kernel for one core: d DVE scan tiles + (ROWS - 128d) PE rows.
    repeats>1 re-runs the sweep inside one NEFF for timing harnesses.
    pair_in: load two PE seq-chunks per DMA (3D AP) to halve SP dispatches."""
    r_pe = ROWS - d * P
    half = r_pe // 2
    assert half <= 512 and r_pe % 2 == 0
    nc = bacc.Bacc(trn_type="TRN2")
    x = nc.dram_tensor("x", [max(d * P, 1), L], FP16, kind="ExternalInput")
    trend = nc.dram_tensor("trend", [max(d * P, 1), L], FP16,
                           kind="ExternalOutput")
    if r_pe:
        xt = nc.dram_tensor("xt", [L, r_pe], FP16, kind="ExternalInput")
        wdr = {n: nc.dram_tensor(n, list(WSHAPES[n]), FP16,
                                 kind="ExternalInput") for n in WNAMES}
        tt = nc.dram_tensor("trendt", [L, r_pe], FP16, kind="ExternalOutput")

    with TileContext(nc) as tc:
        with tc.tile_pool(name="wpool", bufs=1) as wpool, \
             tc.tile_pool(name="dpool", bufs=6) as dpool, \
             tc.tile_pool(name="ppool", bufs=4) as ppool, \
             tc.tile_pool(name="psum", bufs=4,
                          space=bass.MemorySpace.PSUM) as psum:
            wsb = {}
            if r_pe:
                for n in WNAMES:
                    wt = wpool.tile(list(WSHAPES[n]), FP16, tag=f"t_{n}")
                    nc.sync.dma_start(out=wt[:, :], in_=wdr[n][:, :])
                    wsb[n] = wt
            for rep in range(repeats):
                # All 32 PE chunk tiles stay resident (they fit in SBUF), so
                # PE matmuls never stall the SP queue on input. Their loads
                # are interleaved with the DVE tiles so the DVE path starts
                # immediately; PE compute waits on semaphores, not the queue.
                xtiles = []
                npair = NCH // 2 if r_pe else 0
                nstep = max(d, npair)
                pairs_done = 0

                if pair_in:
                    def chunkref(k):
                        return xtiles[k // 2], (k % 2) * r_pe

                    def _load_pe(upto):
                        # one DMA covers two seq-chunks: partition p holds
                        # DRAM rows 256j+p and 256j+128+p side by side
                        while r_pe and len(xtiles) < min((upto + 1) // 2,
                                                         npair):
                            j = len(xtiles)
                            xtile = ppool.tile([P, 2 * r_pe], FP16, tag="xt",
                                               bufs=npair)
                            nc.sync.dma_start(
                                out=xtile[:, :],
                                in_=xt[2 * j * P : (2 * j + 2) * P, :]
                                .rearrange("(a p) b -> p a b", p=P),
                            )
                            xtiles.append(xtile)

                    def loaded_chunks():
                        return 2 * len(xtiles)
                else:
                    def chunkref(k):
                        return xtiles[k], 0

                    def _load_pe(upto):
                        while r_pe and len(xtiles) < min(upto, NCH):
                            k = len(xtiles)
                            xtile = ppool.tile([P, r_pe], FP16, tag="xt",
                                               bufs=NCH)
                            nc.sync.dma_start(
                                out=xtile[:, :], in_=xt[k * P : (k + 1) * P, :]
                            )
                            xtiles.append(xtile)

                    def loaded_chunks():
                        return len(xtiles)

                for i in range(nstep):
                    if i < d:
                        _load_pe((i + 1) * NCH // max(d, 1))
                        _emit_dve_tile(nc, dpool, x, trend, i, L)
                    _load_pe(NCH if i >= d - 1 else 0)
                    while r_pe and pairs_done < min((i + 1) * npair // nstep
                                                    if nstep > d else i + 1,
                                                    npair):
                        j = pairs_done
                        if loaded_chunks() >= min(2 * j + 3, NCH):
                            _emit_pe_pair(nc, ppool, psum, wsb, tt, r_pe,
                                          half, j, chunkref)
                            pairs_done += 1
                        else:
                            break
                while r_pe and pairs_done < npair:
                    _emit_pe_pair(nc, ppool, psum, wsb, tt, r_pe, half,
                                  pairs_done, chunkref)
                    pairs_done += 1
    nc.finalize()
    return nc


def _probe_devices():
    """Touch every NeuronCore with a trivial computation to clear stale
    device state from a previous client."""
    try:
        import jax
        import jax.numpy as jnp

        for dev in jax.devices():
            y = jax.device_put(np.ones((4, 4), np.float32), dev)
            jnp.sum(y).block_until_ready()
    except Exception:
        pass


def kernel(x, weight):
    x = np.ascontiguousarray(np.asarray(x), dtype=np.float32)
    scale = float(np.asarray(weight).reshape(-1)[0])
    xs = (x.reshape(NCORES, ROWS, L) * scale).astype(np.float16)
    wts = band_weights()
    nc = build_nc()
    nd = D_DVE * P
    in_maps = []
    for c in range(NCORES):
        m = {"x": np.ascontiguousarray(xs[c, :nd])}
        if R_PE:
            m["xt"] = np.ascontiguousarray(xs[c, nd:].T)
            m.update(wts)
        in_maps.append(m)
    _probe_devices()
    out = None
    for attempt in range(3):
        try:
            out = run_bass_kernel_spmd(nc, in_maps, core_ids=list(range(NCORES)))
            break
        except Exception:
            if attempt == 2:
                raise
            try:
                import jax

                jax.clear_backends()
            except Exception:
                pass
            _probe_devices()
    trend = np.empty((NCORES, ROWS, L), np.float32)
    for c in range(NCORES):
        trend[c, :nd] = out.results[c]["trend"]
        if R_PE:
            trend[c, nd:] = out.results[c]["trendt"].T
    trend = trend.reshape(B, C, L)
    remainder = x.reshape(B, C, L) - trend
    return trend, remainder
